# revision 19
# baseline (speedup 1.0000x reference)
"""Additive (Bahdanau-style) attention on 8 Trainium2 NeuronCores.

Math: scores[b,q,k] = Wt . tanh(u[b,k] + v[b,q]) + bt, masked softmax over k,
out = weights @ hidden.  (bt dropped: softmax is shift-invariant.)

tanh(x) on |x| <= 10.4 ~= sum_m beta_m sin(om_m x) with a *doubling-ladder*
frequency set {a, b, c, 2a, 2b, 2c, 4b, 4c}: only the 3 base frequencies are
evaluated with the ACT Sin table (all base args stay inside the table's
accurate range |arg| <~ 3.5, so NO range reduction is needed); the other 5
come from double-angle identities on the DVE:
    sin(2w) = 2 sin(w) cos(w)   (factor 2 deferred into the beta*Wt scale)
    cos(2w) = 1 - 2 sin(w)^2
cos phases come from cos(t) = sin(pi/2 - |t|) (always in table range).
The angle-addition identity sin(om(u+v)) = sinU cosV + cosU sinV then
factorizes the [Sq,Sk,A] tanh tensor into per-(a,k)/per-(a,q) f16 feature
maps plus PE matmuls contracting over A.

Scores are accumulated TRANSPOSED ([k, q]) so the key mask folds into the
softmax Exp's per-partition bias for free, and the exp weights feed the
value matmuls directly as stationary operands (no transposes/copies).

Sharding: core c -> batch b = c//2, query half qoff = (c%2)*256.  Each core's
inputs are rolled by -qoff so queries are always rows 0..255 (pure SPMD);
a key permutation is softmax-invariant when mask and values are permuted
consistently.
"""

import numpy as np

import concourse.bass as bass
import concourse.tile as tile
from concourse import bacc, mybir
from concourse.bass_utils import run_bass_kernel_spmd

# ---- problem constants (hardcoded; kernel.py must be self-contained) -------
B, S, D, A = 4, 512, 256, 128
QPC = 256          # queries per core
NCORES = 8
MASK_NEG = -30000.0
HALF_PI = float(np.pi / 2)

# ---- tanh ~= sum_m beta_m sin(om_m x), doubling ladder ---------------------
# m: 0..2 = bases {a,b,c} (ACT Sin), 3..5 = doubles {2a,2b,2c}, 6..7 = {4b,4c}
OMEGA = [0.19661398572560465, 0.7067266116223141, 0.992731100727015,
         0.3932279714512093, 1.4134532232446282, 1.98546220145403,
         2.8269064464892564, 3.97092440290806]
BETA = [1.0701015472411093, 0.2252177841876598, 0.1527837415896906,
        0.3031134355844352, 0.10362678834616496, 0.06220737550216762,
        0.0230515652310954, 0.005206629732025753]
# stored sin tiles carry a deferred factor (2 per doubling), folded into bwt
DEFER = [1.0, 1.0, 1.0, 2.0, 2.0, 2.0, 4.0, 4.0]
M = 8
NBASE = 3
W = S + QPC                      # concat feature width: [u(0:512) | v(512:768)]

NH_INIT = 10                     # PE heater matmuls during the DMA/proj fill
NH_MID = 14                      # heaters covering the Sin/feature gap

TRACE = False                    # test.py sets True for the profiled run
LAST_EXEC_NS = None


def _ensure_ntff_hook():
    """The agent image's `antenv` lacks `axon_hooks`; recreate the NTFF
    profiling hook registration (stub module wired to the ctypes profiler)."""
    import sys, types
    if "antenv.axon_hooks" in sys.modules:
        return
    mod = types.ModuleType("antenv.axon_hooks")
    _h = [None]
    mod.set_axon_ntff_profile_hook = lambda h: _h.__setitem__(0, h)
    mod.get_axon_ntff_profile_hook = lambda: _h[0]
    import antenv
    sys.modules["antenv.axon_hooks"] = mod
    antenv.axon_hooks = mod
    try:
        from trn_agent_boot.trn_boot import _ntff_profile_via_ctypes
        mod.set_axon_ntff_profile_hook(
            _ntff_profile_via_ctypes("/opt/axon/libaxon_pjrt.so"))
    except Exception:
        pass


# ---- device program --------------------------------------------------------
_NC = None


def _build_program():
    f32 = mybir.dt.float32
    f16 = mybir.dt.float16
    nc = bacc.Bacc("TRN2", target_bir_lowering=False, debug=False,
                   num_devices=NCORES)

    ht_ext = nc.dram_tensor("ht16", [D, S], f16, kind="ExternalInput").ap()
    h_ext = nc.dram_tensor("h16", [S, D], f16, kind="ExternalInput").ap()
    wut_ext = nc.dram_tensor("wut16", [D, A], f16, kind="ExternalInput").ap()
    wvt_ext = nc.dram_tensor("wvt16", [D, A], f16, kind="ExternalInput").ap()
    bu_ext = nc.dram_tensor("bu16", [1, A], f16, kind="ExternalInput").ap()
    mbt_ext = nc.dram_tensor("mbt", [A, 4], f32, kind="ExternalInput").ap()
    bwt_ext = nc.dram_tensor("bwt8", [A, M], f32, kind="ExternalInput").ap()
    out_ext = nc.dram_tensor("out", [QPC, D], f32, kind="ExternalOutput").ap()

    from concourse.masks import make_identity
    P = 128
    SIN = mybir.ActivationFunctionType.Sin
    EXP = mybir.ActivationFunctionType.Exp
    COPY = mybir.ActivationFunctionType.Copy
    SQUARE = mybir.ActivationFunctionType.Square
    ALU = mybir.AluOpType

    with tile.TileContext(nc) as tc:
        import contextlib
        with contextlib.ExitStack() as ctx:
            const = ctx.enter_context(tc.tile_pool(name="const", bufs=1))
            feat = ctx.enter_context(tc.tile_pool(name="feat", bufs=1))
            wrk = ctx.enter_context(tc.tile_pool(name="wrk", bufs=1))
            pp_sc = ctx.enter_context(
                tc.tile_pool(name="pp_sc", bufs=1, space="PSUM"))
            pp_uv = ctx.enter_context(
                tc.tile_pool(name="pp_uv", bufs=1, space="PSUM"))
            pp_sm = ctx.enter_context(
                tc.tile_pool(name="pp_sm", bufs=1, space="PSUM"))

            # ---- constants ----
            idf = const.tile([P, P], f16)
            make_identity(nc, idf)
            ones_row = const.tile([1, QPC], f16)
            nc.vector.memset(ones_row, 1.0)
            ones_col = const.tile([P, 1], f16)
            nc.vector.memset(ones_col, 1.0)
            zrow = const.tile([1, P], f16)
            nc.vector.memset(zrow, 0.0)
            zb = const.tile([P, 1], mybir.dt.float32)
            nc.vector.memset(zb, 0.0)
            hp = const.tile([P, 1], mybir.dt.float32)
            nc.vector.memset(hp, HALF_PI)

            # ---- inputs ----
            wvT = const.tile([P, 2, A], f16)
            wuT = const.tile([P, 2, A], f16)
            hT = const.tile([P, 2, S], f16)      # [d_p, d_chunk, s]
            h_sb = const.tile([P, 4, D], f16)    # [k_p, k_chunk, d]
            bu_sb = const.tile([1, A], f16)
            mbt = const.tile([P, 4], f32)
            bwt = const.tile([P, M], f32)
            ht_r = ht_ext.rearrange("(c p) s -> p c s", p=P)
            h_r = h_ext.rearrange("(t p) d -> p t d", p=P)
            # few big DMAs, one per engine queue (DGE issue cost dominates)
            nc.sync.dma_start(out=wvT,
                              in_=wvt_ext.rearrange("(c p) a -> p c a", p=P))
            nc.sync.dma_start(out=wuT,
                              in_=wut_ext.rearrange("(c p) a -> p c a", p=P))
            nc.scalar.dma_start(out=bu_sb, in_=bu_ext[:])

            nc.scalar.dma_start(out=bwt, in_=bwt_ext[:])
            nc.scalar.dma_start(out=mbt, in_=mbt_ext[:])
            nc.sync.dma_start(out=hT[:, 0, :], in_=ht_r[:, 0, :])
            nc.sync.dma_start(out=hT[:, 1, :], in_=ht_r[:, 1, :])
            nc.scalar.dma_start(out=h_sb[:, 0:2, :], in_=h_r[:, 0:2, :])
            nc.scalar.dma_start(out=h_sb[:, 2:4, :], in_=h_r[:, 2:4, :])

            # ---- psum tiles ----
            # one accumulation group per PSUM bank: a matmul group-start
            # resets the whole bank, so concurrent groups must not share one
            psT = pp_sc.tile([P, 4, 2 * QPC], f32)  # scores^T, kb -> own bank
            ps_uv = pp_uv.tile([P, 4 * QPC], f32, tag="uv")  # u bank0 | v bank1
            ps_sm = pp_sm.tile([P, 3, P], f32)      # sums cols + heater area

            # PE heater: garbage matmuls keep the clock-ramp window alive
            # while PE waits for DMA / feature maps.  Round-robin over the
            # unused upper halves of the score banks so they overlap; the
            # m=0 group-start resets those banks afterwards anyway.
            _hb = [0]
            def heat(n, dep):
                w = min(dep.shape[-1], P)
                for i in range(n):
                    _hb[0] = (_hb[0] + 1) % 4
                    nc.tensor.matmul(psT[0:w, _hb[0], QPC:QPC + w],
                                     dep[:, 0:w], dep[:, 0:w],
                                     start=False, stop=False,
                                     skip_group_check=True)

            heat(NH_INIT, idf)

            # ---- projections: ps_v = Wv h^T (+bu), ps_u = Wu h^T ----
            ps_v = ps_uv[:, 2 * QPC:3 * QPC]
            ps_u = ps_uv[:, 0:2 * QPC]
            nc.tensor.matmul(ps_v, bu_sb, ones_row, start=True, stop=False)
            for c in range(2):
                nc.tensor.matmul(ps_v, wvT[:, c, :], hT[:, c, 0:QPC],
                                 start=False, stop=(c == 1))
            for c in range(2):
                nc.tensor.matmul(ps_u, wuT[:, c, :], hT[:, c, :],
                                 start=(c == 0), stop=(c == 1))
            heat(NH_MID, idf)

            # ---- f16 copies + |.| ----
            uv = wrk.tile([P, W], f16)
            au = wrk.tile([P, W], f16)
            # f16 + clamp to +-4.2: keeps all base Sin args inside the ACT
            # table's accurate range (|arg| <= 4.2*0.993 ~ 4.17)
            nc.vector.tensor_scalar(out=uv[:, 0:S], in0=ps_u, scalar1=4.2,
                                    scalar2=-4.2, op0=ALU.min, op1=ALU.max)
            nc.vector.tensor_scalar(out=uv[:, S:W], in0=ps_v, scalar1=4.2,
                                    scalar2=-4.2, op0=ALU.min, op1=ALU.max)
            nc.vector.scalar_tensor_tensor(out=au, in0=uv, scalar=-1.0,
                                           in1=uv, op0=ALU.mult, op1=ALU.max)

            # ---- features: F[:, m, 0, :]=sin-ish, F[:, m, 1, :]=cos ----
            F = feat.tile([P, M, 2, W], f16)
            SQ = feat.tile([P, 6, W], f16)
            VS = feat.tile([P, M, 2, QPC], f16)

            def vscale(m):
                # v-side scale by beta*defer*Wt (gpsimd is ~30x too slow here)
                nc.vector.tensor_scalar(out=VS[:, m, :, :],
                                        in0=F[:, m, :, S:W],
                                        scalar1=bwt[:, m:m + 1], scalar2=None,
                                        op0=ALU.mult)

            def base_sins(i):
                nc.scalar.activation(F[:, i, 0, :], uv, SIN,
                                     bias=zb, scale=float(OMEGA[i]))
                nc.scalar.activation(F[:, i, 1, :], au, SIN,
                                     bias=hp, scale=float(-OMEGA[i]))

            def dbl(i, j, cc, sq_act=False):
                # derive tile i = double of tile j (cc = deferred factor of j)
                if sq_act:
                    nc.scalar.activation(SQ[:, j, :], F[:, j, 0, :], SQUARE,
                                         bias=zb, scale=1.0)
                else:
                    nc.vector.tensor_tensor(out=SQ[:, j, :],
                                            in0=F[:, j, 0, :],
                                            in1=F[:, j, 0, :], op=ALU.mult)
                nc.vector.tensor_tensor(out=F[:, i, 0, :], in0=F[:, j, 0, :],
                                        in1=F[:, j, 1, :], op=ALU.mult)
                nc.vector.tensor_scalar(out=F[:, i, 1, :], in0=SQ[:, j, :],
                                        scalar1=-2.0 * cc * cc, scalar2=1.0,
                                        op0=ALU.mult, op1=ALU.add)

            # deep-chain bases first so the DVE ladder starts early
            base_sins(1)
            vscale(1)
            dbl(4, 1, 1.0)        # 2b
            vscale(4)
            base_sins(2)
            dbl(6, 4, 2.0, sq_act=True)   # 4b
            vscale(6)
            vscale(2)
            dbl(5, 2, 1.0)        # 2c
            vscale(5)
            base_sins(0)
            dbl(7, 5, 2.0, sq_act=True)   # 4c
            vscale(7)
            vscale(0)
            dbl(3, 0, 1.0, sq_act=True)   # 2a
            vscale(3)

            # ---- scores^T: psT[k, q] += sinU^T (bwt cosV) + cosU^T (bwt sinV)
            MS_ORDER = [1, 4, 6, 2, 5, 7, 0, 3]
            for mi, m in enumerate(MS_ORDER):
                for kb in range(4):
                    ks = slice(kb * P, (kb + 1) * P)
                    nc.tensor.matmul(psT[:, kb, 0:QPC], F[:, m, 0, ks],
                                     VS[:, m, 1, :],
                                     start=(mi == 0), stop=False)
                    nc.tensor.matmul(psT[:, kb, 0:QPC], F[:, m, 1, ks],
                                     VS[:, m, 0, :],
                                     start=False, stop=(mi == M - 1))

            # ---- masked softmax (mask = exp bias) + value matmuls ----
            expT = wrk.tile([P, 4, QPC], f16)
            for kb in range(4):
                nc.scalar.activation(expT[:, kb, :], psT[:, kb, 0:QPC], EXP,
                                     bias=mbt[:, kb:kb + 1], scale=1.0)
            ps_o = pp_uv.tile([P, 2, 2 * D], f32, tag="uv")
            # seed the sums bank once; both qb groups then accumulate
            nc.tensor.matmul(ps_sm[:, 0, 0:2], zrow, ones_row[:, 0:2],
                             start=True, stop=False, skip_group_check=True)
            for qb in range(2):
                qs = slice(qb * P, (qb + 1) * P)
                for kb in range(4):
                    nc.tensor.matmul(ps_o[:, qb, 0:D], expT[:, kb, qs],
                                     h_sb[:, kb, :],
                                     start=(kb == 0), stop=(kb == 3))
                    nc.tensor.matmul(ps_sm[:, 0, qb:qb + 1], expT[:, kb, qs],
                                     ones_col, start=False,
                                     stop=(qb == 1 and kb == 3),
                                     skip_group_check=True)
            rs = wrk.tile([P, 2], f32)
            nc.vector.reciprocal(rs, ps_sm[:, 0, 0:2])
            out_sb = wrk.tile([P, 2, D], f32)
            for qb in range(2):
                nc.vector.tensor_scalar(out=out_sb[:, qb, :],
                                        in0=ps_o[:, qb, 0:D],
                                        scalar1=rs[:, qb:qb + 1], scalar2=None,
                                        op0=ALU.mult)
                oq = nc.sync if qb == 0 else nc.scalar
                oq.dma_start(out=out_ext[qb * P:(qb + 1) * P, :],
                             in_=out_sb[:, qb, :])

    nc.compile()
    return nc


def kernel(hidden, mask, Wu, bu, Wv, Wt, bt):
    global _NC, LAST_EXEC_NS
    if _NC is None:
        _NC = _build_program()
    nc = _NC

    hidden = np.asarray(hidden, dtype=np.float32)
    mask = np.asarray(mask)
    Wu = np.asarray(Wu, dtype=np.float32)
    Wv = np.asarray(Wv, dtype=np.float32)
    Wt_v = np.asarray(Wt, dtype=np.float32).reshape(A)
    bu_v = np.asarray(bu, dtype=np.float32).reshape(A)

    WuT16 = np.ascontiguousarray(Wu.T.astype(np.float16))
    WvT16 = np.ascontiguousarray(Wv.T.astype(np.float16))
    bu16 = np.ascontiguousarray(bu_v.reshape(1, A).astype(np.float16))
    bwt8 = np.ascontiguousarray(
        (np.asarray(BETA) * np.asarray(DEFER))[None, :] * Wt_v[:, None]
    ).astype(np.float32)                                   # [A, M]

    in_maps = []
    for c in range(NCORES):
        b, half = divmod(c, 2)
        qoff = half * QPC
        hr = np.roll(hidden[b], -qoff, axis=0)
        h16 = np.ascontiguousarray(hr.astype(np.float16))
        ht16 = np.ascontiguousarray(hr.T.astype(np.float16))
        mb = np.where(np.asarray(mask[b]) < 1, MASK_NEG, 0.0).astype(np.float32)
        mbr = np.roll(mb, -qoff)                           # [S]
        mbt = np.ascontiguousarray(mbr.reshape(4, A).T)    # [A, 4] col = kblk
        in_maps.append({"h16": h16, "ht16": ht16, "wut16": WuT16,
                        "wvt16": WvT16, "bu16": bu16, "mbt": mbt,
                        "bwt8": bwt8})
    if TRACE:
        _ensure_ntff_hook()
    res = run_bass_kernel_spmd(nc, in_maps, list(range(NCORES)), trace=TRACE)
    LAST_EXEC_NS = res.exec_time_ns

    out = np.empty((B, S, D), dtype=np.float32)
    for c in range(NCORES):
        b, half = divmod(c, 2)
        qoff = half * QPC
        out[b, qoff:qoff + QPC] = res.results[c]["out"]
    return out


# revision 20
# speedup vs baseline: 1.0343x; 1.0343x over previous
"""Additive (Bahdanau-style) attention on 8 Trainium2 NeuronCores.

Math: scores[b,q,k] = Wt . tanh(u[b,k] + v[b,q]) + bt, masked softmax over k,
out = weights @ hidden.  (bt dropped: softmax is shift-invariant.)

tanh(x) on |x| <= 10.4 ~= sum_m beta_m sin(om_m x) with a *doubling-ladder*
frequency set {a, b, c, 2a, 2b, 2c, 4b, 4c}: only the 3 base frequencies are
evaluated with the ACT Sin table (all base args stay inside the table's
accurate range |arg| <~ 3.5, so NO range reduction is needed); the other 5
come from double-angle identities on the DVE:
    sin(2w) = 2 sin(w) cos(w)   (factor 2 deferred into the beta*Wt scale)
    cos(2w) = 1 - 2 sin(w)^2
cos phases come from cos(t) = sin(pi/2 - |t|) (always in table range).
The angle-addition identity sin(om(u+v)) = sinU cosV + cosU sinV then
factorizes the [Sq,Sk,A] tanh tensor into per-(a,k)/per-(a,q) f16 feature
maps plus PE matmuls contracting over A.

Scores are accumulated TRANSPOSED ([k, q]) so the key mask folds into the
softmax Exp's per-partition bias for free, and the exp weights feed the
value matmuls directly as stationary operands (no transposes/copies).

Sharding: core c -> batch b = c//2, query half qoff = (c%2)*256.  Each core's
inputs are rolled by -qoff so queries are always rows 0..255 (pure SPMD);
a key permutation is softmax-invariant when mask and values are permuted
consistently.
"""

import numpy as np

import concourse.bass as bass
import concourse.tile as tile
from concourse import bacc, mybir
from concourse.bass_utils import run_bass_kernel_spmd

# ---- problem constants (hardcoded; kernel.py must be self-contained) -------
B, S, D, A = 4, 512, 256, 128
QPC = 256          # queries per core
NCORES = 8
MASK_NEG = -30000.0
HALF_PI = float(np.pi / 2)

# ---- tanh ~= sum_m beta_m sin(om_m x), doubling ladder ---------------------
# m: 0..2 = bases {a,b,c} (ACT Sin), 3..5 = doubles {2a,2b,2c}, 6..7 = {4b,4c}
OMEGA = [0.19661398572560465, 0.7067266116223141, 0.992731100727015,
         0.3932279714512093, 1.4134532232446282, 1.98546220145403,
         2.8269064464892564, 3.97092440290806]
BETA = [1.0701015472411093, 0.2252177841876598, 0.1527837415896906,
        0.3031134355844352, 0.10362678834616496, 0.06220737550216762,
        0.0230515652310954, 0.005206629732025753]
# stored sin tiles carry a deferred factor (2 per doubling), folded into bwt
DEFER = [1.0, 1.0, 1.0, 2.0, 2.0, 2.0, 4.0, 4.0]
M = 8
NBASE = 3
W = S + QPC                      # concat feature width: [u(0:512) | v(512:768)]

NH_INIT = 10                     # PE heater matmuls during the DMA/proj fill
NH_MID = 14                      # heaters covering the Sin/feature gap

TRACE = False                    # test.py sets True for the profiled run
LAST_EXEC_NS = None


def _ensure_ntff_hook():
    """The agent image's `antenv` lacks `axon_hooks`; recreate the NTFF
    profiling hook registration (stub module wired to the ctypes profiler)."""
    import sys, types
    if "antenv.axon_hooks" in sys.modules:
        return
    mod = types.ModuleType("antenv.axon_hooks")
    _h = [None]
    mod.set_axon_ntff_profile_hook = lambda h: _h.__setitem__(0, h)
    mod.get_axon_ntff_profile_hook = lambda: _h[0]
    import antenv
    sys.modules["antenv.axon_hooks"] = mod
    antenv.axon_hooks = mod
    try:
        from trn_agent_boot.trn_boot import _ntff_profile_via_ctypes
        mod.set_axon_ntff_profile_hook(
            _ntff_profile_via_ctypes("/opt/axon/libaxon_pjrt.so"))
    except Exception:
        pass


# ---- device program --------------------------------------------------------
_NC = None


def _build_program():
    f32 = mybir.dt.float32
    f16 = mybir.dt.float16
    nc = bacc.Bacc("TRN2", target_bir_lowering=False, debug=False,
                   num_devices=NCORES)

    ht_ext = nc.dram_tensor("ht16", [D, S], f16, kind="ExternalInput").ap()
    h_ext = nc.dram_tensor("h16", [S, D], f16, kind="ExternalInput").ap()
    wut_ext = nc.dram_tensor("wut16", [D, A], f16, kind="ExternalInput").ap()
    wvt_ext = nc.dram_tensor("wvt16", [D, A], f16, kind="ExternalInput").ap()
    bu_ext = nc.dram_tensor("bu16", [1, A], f16, kind="ExternalInput").ap()
    mbt_ext = nc.dram_tensor("mbt", [A, 4], f32, kind="ExternalInput").ap()
    bwt_ext = nc.dram_tensor("bwt8", [A, M], f32, kind="ExternalInput").ap()
    out_ext = nc.dram_tensor("out", [QPC, D], f32, kind="ExternalOutput").ap()

    from concourse.masks import make_identity
    P = 128
    SIN = mybir.ActivationFunctionType.Sin
    EXP = mybir.ActivationFunctionType.Exp
    COPY = mybir.ActivationFunctionType.Copy
    SQUARE = mybir.ActivationFunctionType.Square
    ALU = mybir.AluOpType

    with tile.TileContext(nc) as tc:
        import contextlib
        with contextlib.ExitStack() as ctx:
            const = ctx.enter_context(tc.tile_pool(name="const", bufs=1))
            feat = ctx.enter_context(tc.tile_pool(name="feat", bufs=1))
            wrk = ctx.enter_context(tc.tile_pool(name="wrk", bufs=1))
            pp_sc = ctx.enter_context(
                tc.tile_pool(name="pp_sc", bufs=1, space="PSUM"))
            pp_uv = ctx.enter_context(
                tc.tile_pool(name="pp_uv", bufs=1, space="PSUM"))
            pp_sm = ctx.enter_context(
                tc.tile_pool(name="pp_sm", bufs=1, space="PSUM"))

            # ---- constants ----
            idf = const.tile([P, P], f16)
            make_identity(nc, idf)
            ones_row = const.tile([1, QPC], f16)
            nc.vector.memset(ones_row, 1.0)
            ones_col = const.tile([P, 1], f16)
            nc.vector.memset(ones_col, 1.0)
            zrow = const.tile([1, P], f16)
            nc.vector.memset(zrow, 0.0)
            zb = const.tile([P, 1], mybir.dt.float32)
            nc.vector.memset(zb, 0.0)
            hp = const.tile([P, 1], mybir.dt.float32)
            nc.vector.memset(hp, HALF_PI)

            # ---- inputs ----
            wvT = const.tile([P, 2, A], f16)
            wuT = const.tile([P, 2, A], f16)
            hT = const.tile([P, 2, S], f16)      # [d_p, d_chunk, s]
            h_sb = const.tile([P, 4, D], f16)    # [k_p, k_chunk, d]
            bu_sb = const.tile([1, A], f16)
            mbt = const.tile([P, 4], f32)
            bwt = const.tile([P, M], f32)
            ht_r = ht_ext.rearrange("(c p) s -> p c s", p=P)
            h_r = h_ext.rearrange("(t p) d -> p t d", p=P)
            # few big DMAs, one per engine queue (DGE issue cost dominates)
            nc.sync.dma_start(out=wvT,
                              in_=wvt_ext.rearrange("(c p) a -> p c a", p=P))
            nc.sync.dma_start(out=wuT,
                              in_=wut_ext.rearrange("(c p) a -> p c a", p=P))
            nc.scalar.dma_start(out=bu_sb, in_=bu_ext[:])

            nc.scalar.dma_start(out=bwt, in_=bwt_ext[:])
            nc.scalar.dma_start(out=mbt, in_=mbt_ext[:])
            nc.sync.dma_start(out=hT[:, 0, :], in_=ht_r[:, 0, :])
            nc.sync.dma_start(out=hT[:, 1, :], in_=ht_r[:, 1, :])
            nc.scalar.dma_start(out=h_sb[:, 0:2, :], in_=h_r[:, 0:2, :])
            nc.scalar.dma_start(out=h_sb[:, 2:4, :], in_=h_r[:, 2:4, :])

            # ---- psum tiles ----
            # one accumulation group per PSUM bank: a matmul group-start
            # resets the whole bank, so concurrent groups must not share one
            psT = pp_sc.tile([P, 4, 2 * QPC], f32)  # scores^T, kb -> own bank
            ps_uv = pp_uv.tile([P, 4 * QPC], f32, tag="uv")  # u bank0 | v bank1
            ps_sm = pp_sm.tile([P, 3, P], f32)      # sums cols + heater area

            # PE heater: garbage matmuls keep the clock-ramp window alive
            # while PE waits for DMA / feature maps.  Round-robin over the
            # unused upper halves of the score banks so they overlap; the
            # m=0 group-start resets those banks afterwards anyway.
            _hb = [0]
            def heat(n, dep):
                w = min(dep.shape[-1], P)
                for i in range(n):
                    _hb[0] = (_hb[0] + 1) % 4
                    nc.tensor.matmul(psT[0:w, _hb[0], QPC:QPC + w],
                                     dep[:, 0:w], dep[:, 0:w],
                                     start=False, stop=False,
                                     skip_group_check=True)

            heat(NH_INIT, idf)

            # ---- projections: ps_v = Wv h^T (+bu), ps_u = Wu h^T ----
            ps_v = ps_uv[:, 2 * QPC:3 * QPC]
            ps_u = ps_uv[:, 0:2 * QPC]
            nc.tensor.matmul(ps_v, bu_sb, ones_row, start=True, stop=False)
            for c in range(2):
                nc.tensor.matmul(ps_v, wvT[:, c, :], hT[:, c, 0:QPC],
                                 start=False, stop=(c == 1))
            for c in range(2):
                nc.tensor.matmul(ps_u, wuT[:, c, :], hT[:, c, :],
                                 start=(c == 0), stop=(c == 1))
            heat(NH_MID, idf)

            # ---- f16 copies + |.| ----
            uv = wrk.tile([P, W], f16)
            au = wrk.tile([P, W], f16)
            # f16 + clamp to +-4.2: keeps all base Sin args inside the ACT
            # table's accurate range (|arg| <= 4.2*0.993 ~ 4.17)
            nc.vector.tensor_scalar(out=uv[:, 0:S], in0=ps_u, scalar1=4.2,
                                    scalar2=-4.2, op0=ALU.min, op1=ALU.max)
            nc.vector.tensor_scalar(out=uv[:, S:W], in0=ps_v, scalar1=4.2,
                                    scalar2=-4.2, op0=ALU.min, op1=ALU.max)
            nc.vector.scalar_tensor_tensor(out=au, in0=uv, scalar=-1.0,
                                           in1=uv, op0=ALU.mult, op1=ALU.max)

            # ---- features: F[:, m, 0, :]=sin-ish, F[:, m, 1, :]=cos ----
            F = feat.tile([P, M, 2, W], f16)
            SQ = feat.tile([P, 6, W], f16)
            VS = feat.tile([P, M, 2, QPC], f16)

            def vscale(m):
                # v-side scale by beta*defer*Wt (gpsimd is ~30x too slow here)
                nc.vector.tensor_scalar(out=VS[:, m, :, :],
                                        in0=F[:, m, :, S:W],
                                        scalar1=bwt[:, m:m + 1], scalar2=None,
                                        op0=ALU.mult)

            def base_sins(i):
                nc.scalar.activation(F[:, i, 0, :], uv, SIN,
                                     bias=zb, scale=float(OMEGA[i]))
                nc.scalar.activation(F[:, i, 1, :], au, SIN,
                                     bias=hp, scale=float(-OMEGA[i]))

            def dbl(i, j, cc, sq_act=False):
                # derive tile i = double of tile j (cc = deferred factor of j)
                if sq_act:
                    nc.scalar.activation(SQ[:, j, :], F[:, j, 0, :], SQUARE,
                                         bias=zb, scale=1.0)
                else:
                    nc.vector.tensor_tensor(out=SQ[:, j, :],
                                            in0=F[:, j, 0, :],
                                            in1=F[:, j, 0, :], op=ALU.mult)
                nc.vector.tensor_tensor(out=F[:, i, 0, :], in0=F[:, j, 0, :],
                                        in1=F[:, j, 1, :], op=ALU.mult)
                nc.vector.tensor_scalar(out=F[:, i, 1, :], in0=SQ[:, j, :],
                                        scalar1=-2.0 * cc * cc, scalar2=1.0,
                                        op0=ALU.mult, op1=ALU.add)

            # deep-chain bases first so the DVE ladder starts early
            base_sins(1)
            vscale(1)
            dbl(4, 1, 1.0)        # 2b
            vscale(4)
            base_sins(2)
            dbl(6, 4, 2.0)   # 4b
            vscale(6)
            vscale(2)
            dbl(5, 2, 1.0)        # 2c
            vscale(5)
            base_sins(0)
            dbl(7, 5, 2.0)   # 4c
            vscale(7)
            vscale(0)
            dbl(3, 0, 1.0)   # 2a
            vscale(3)

            # ---- scores^T: psT[k, q] += sinU^T (bwt cosV) + cosU^T (bwt sinV)
            MS_ORDER = [1, 4, 6, 2, 5, 7, 0, 3]
            for mi, m in enumerate(MS_ORDER):
                for kb in range(4):
                    ks = slice(kb * P, (kb + 1) * P)
                    nc.tensor.matmul(psT[:, kb, 0:QPC], F[:, m, 0, ks],
                                     VS[:, m, 1, :],
                                     start=(mi == 0), stop=False)
                    nc.tensor.matmul(psT[:, kb, 0:QPC], F[:, m, 1, ks],
                                     VS[:, m, 0, :],
                                     start=False, stop=(mi == M - 1))

            # ---- masked softmax (mask = exp bias) + value matmuls ----
            expT = wrk.tile([P, 4, QPC], f16)
            for kb in range(4):
                nc.scalar.activation(expT[:, kb, :], psT[:, kb, 0:QPC], EXP,
                                     bias=mbt[:, kb:kb + 1], scale=1.0)
            ps_o = pp_uv.tile([P, 2, 2 * D], f32, tag="uv")
            # seed the sums bank once; both qb groups then accumulate
            nc.tensor.matmul(ps_sm[:, 0, 0:2], zrow, ones_row[:, 0:2],
                             start=True, stop=False, skip_group_check=True)
            for qb in range(2):
                qs = slice(qb * P, (qb + 1) * P)
                for kb in range(4):
                    nc.tensor.matmul(ps_o[:, qb, 0:D], expT[:, kb, qs],
                                     h_sb[:, kb, :],
                                     start=(kb == 0), stop=(kb == 3))
                    nc.tensor.matmul(ps_sm[:, 0, qb:qb + 1], expT[:, kb, qs],
                                     ones_col, start=False,
                                     stop=(qb == 1 and kb == 3),
                                     skip_group_check=True)
            rs = wrk.tile([P, 2], f32)
            nc.vector.reciprocal(rs, ps_sm[:, 0, 0:2])
            out_sb = wrk.tile([P, 2, D], f32)
            for qb in range(2):
                nc.vector.tensor_scalar(out=out_sb[:, qb, :],
                                        in0=ps_o[:, qb, 0:D],
                                        scalar1=rs[:, qb:qb + 1], scalar2=None,
                                        op0=ALU.mult)
                oq = nc.sync if qb == 0 else nc.scalar
                oq.dma_start(out=out_ext[qb * P:(qb + 1) * P, :],
                             in_=out_sb[:, qb, :])

    nc.compile()
    return nc


def kernel(hidden, mask, Wu, bu, Wv, Wt, bt):
    global _NC, LAST_EXEC_NS
    if _NC is None:
        _NC = _build_program()
    nc = _NC

    hidden = np.asarray(hidden, dtype=np.float32)
    mask = np.asarray(mask)
    Wu = np.asarray(Wu, dtype=np.float32)
    Wv = np.asarray(Wv, dtype=np.float32)
    Wt_v = np.asarray(Wt, dtype=np.float32).reshape(A)
    bu_v = np.asarray(bu, dtype=np.float32).reshape(A)

    WuT16 = np.ascontiguousarray(Wu.T.astype(np.float16))
    WvT16 = np.ascontiguousarray(Wv.T.astype(np.float16))
    bu16 = np.ascontiguousarray(bu_v.reshape(1, A).astype(np.float16))
    bwt8 = np.ascontiguousarray(
        (np.asarray(BETA) * np.asarray(DEFER))[None, :] * Wt_v[:, None]
    ).astype(np.float32)                                   # [A, M]

    in_maps = []
    for c in range(NCORES):
        b, half = divmod(c, 2)
        qoff = half * QPC
        hr = np.roll(hidden[b], -qoff, axis=0)
        h16 = np.ascontiguousarray(hr.astype(np.float16))
        ht16 = np.ascontiguousarray(hr.T.astype(np.float16))
        mb = np.where(np.asarray(mask[b]) < 1, MASK_NEG, 0.0).astype(np.float32)
        mbr = np.roll(mb, -qoff)                           # [S]
        mbt = np.ascontiguousarray(mbr.reshape(4, A).T)    # [A, 4] col = kblk
        in_maps.append({"h16": h16, "ht16": ht16, "wut16": WuT16,
                        "wvt16": WvT16, "bu16": bu16, "mbt": mbt,
                        "bwt8": bwt8})
    if TRACE:
        _ensure_ntff_hook()
    res = run_bass_kernel_spmd(nc, in_maps, list(range(NCORES)), trace=TRACE)
    LAST_EXEC_NS = res.exec_time_ns

    out = np.empty((B, S, D), dtype=np.float32)
    for c in range(NCORES):
        b, half = divmod(c, 2)
        qoff = half * QPC
        out[b, qoff:qoff + QPC] = res.results[c]["out"]
    return out


# revision 21
# speedup vs baseline: 1.1773x; 1.1383x over previous
"""Additive (Bahdanau-style) attention on 8 Trainium2 NeuronCores.

Math: scores[b,q,k] = Wt . tanh(u[b,k] + v[b,q]) + bt, masked softmax over k,
out = weights @ hidden.  (bt dropped: softmax is shift-invariant.)

tanh(x) on |x| <= 10.4 ~= sum_m beta_m sin(om_m x) with a *doubling-ladder*
frequency set {a, b, c, 2a, 2b, 2c, 4b, 4c}: only the 3 base frequencies are
evaluated with the ACT Sin table (all base args stay inside the table's
accurate range |arg| <~ 3.5, so NO range reduction is needed); the other 5
come from double-angle identities on the DVE:
    sin(2w) = 2 sin(w) cos(w)   (factor 2 deferred into the beta*Wt scale)
    cos(2w) = 1 - 2 sin(w)^2
cos phases come from cos(t) = sin(pi/2 - |t|) (always in table range).
The angle-addition identity sin(om(u+v)) = sinU cosV + cosU sinV then
factorizes the [Sq,Sk,A] tanh tensor into per-(a,k)/per-(a,q) f16 feature
maps plus PE matmuls contracting over A.

Scores are accumulated TRANSPOSED ([k, q]) so the key mask folds into the
softmax Exp's per-partition bias for free, and the exp weights feed the
value matmuls directly as stationary operands (no transposes/copies).

Sharding: core c -> batch b = c//2, query half qoff = (c%2)*256.  Each core's
inputs are rolled by -qoff so queries are always rows 0..255 (pure SPMD);
a key permutation is softmax-invariant when mask and values are permuted
consistently.
"""

import numpy as np

import concourse.bass as bass
import concourse.tile as tile
from concourse import bacc, mybir
from concourse.bass_utils import run_bass_kernel_spmd

# ---- problem constants (hardcoded; kernel.py must be self-contained) -------
B, S, D, A = 4, 512, 256, 128
QPC = 256          # queries per core
NCORES = 8
MASK_NEG = -30000.0
HALF_PI = float(np.pi / 2)

# ---- tanh ~= sum_m beta_m sin(om_m x), doubling ladder ---------------------
# m: 0..2 = bases {a,b,c} (ACT Sin), 3..5 = doubles {2a,2b,2c}, 6..7 = {4b,4c}
OMEGA = [0.19661398572560465, 0.7067266116223141, 0.992731100727015,
         0.3932279714512093, 1.4134532232446282, 1.98546220145403,
         2.8269064464892564, 3.97092440290806]
BETA = [1.0701015472411093, 0.2252177841876598, 0.1527837415896906,
        0.3031134355844352, 0.10362678834616496, 0.06220737550216762,
        0.0230515652310954, 0.005206629732025753]
# stored sin tiles carry a deferred factor (2 per doubling), folded into bwt
DEFER = [1.0, 1.0, 1.0, 2.0, 2.0, 2.0, 4.0, 4.0]
M = 8
NBASE = 3
W = S + QPC                      # concat feature width: [u(0:512) | v(512:768)]

NH_INIT = 10                     # PE heater matmuls during the DMA/proj fill
NH_MID = 14                      # heaters covering the Sin/feature gap

TRACE = False                    # test.py sets True for the profiled run
LAST_EXEC_NS = None


def _ensure_ntff_hook():
    """The agent image's `antenv` lacks `axon_hooks`; recreate the NTFF
    profiling hook registration (stub module wired to the ctypes profiler)."""
    import sys, types
    if "antenv.axon_hooks" in sys.modules:
        return
    mod = types.ModuleType("antenv.axon_hooks")
    _h = [None]
    mod.set_axon_ntff_profile_hook = lambda h: _h.__setitem__(0, h)
    mod.get_axon_ntff_profile_hook = lambda: _h[0]
    import antenv
    sys.modules["antenv.axon_hooks"] = mod
    antenv.axon_hooks = mod
    try:
        from trn_agent_boot.trn_boot import _ntff_profile_via_ctypes
        mod.set_axon_ntff_profile_hook(
            _ntff_profile_via_ctypes("/opt/axon/libaxon_pjrt.so"))
    except Exception:
        pass


# ---- device program --------------------------------------------------------
_NC = None


def _build_program():
    f32 = mybir.dt.float32
    f16 = mybir.dt.float16
    nc = bacc.Bacc("TRN2", target_bir_lowering=False, debug=False,
                   num_devices=NCORES)

    ht_ext = nc.dram_tensor("ht16", [D, S], f16, kind="ExternalInput").ap()
    h_ext = nc.dram_tensor("h16", [S, D], f16, kind="ExternalInput").ap()
    wut_ext = nc.dram_tensor("wut16", [D, A], f16, kind="ExternalInput").ap()
    wvt_ext = nc.dram_tensor("wvt16", [D, A], f16, kind="ExternalInput").ap()
    bu_ext = nc.dram_tensor("bu16", [1, A], f16, kind="ExternalInput").ap()
    mbt_ext = nc.dram_tensor("mbt", [A, 4], f32, kind="ExternalInput").ap()
    bwt_ext = nc.dram_tensor("bwt8", [A, M], f32, kind="ExternalInput").ap()
    out_ext = nc.dram_tensor("out", [QPC, D], f32, kind="ExternalOutput").ap()

    from concourse.masks import make_identity
    P = 128
    SIN = mybir.ActivationFunctionType.Sin
    EXP = mybir.ActivationFunctionType.Exp
    COPY = mybir.ActivationFunctionType.Copy
    SQUARE = mybir.ActivationFunctionType.Square
    ALU = mybir.AluOpType

    with tile.TileContext(nc) as tc:
        import contextlib
        with contextlib.ExitStack() as ctx:
            const = ctx.enter_context(tc.tile_pool(name="const", bufs=1))
            feat = ctx.enter_context(tc.tile_pool(name="feat", bufs=1))
            wrk = ctx.enter_context(tc.tile_pool(name="wrk", bufs=1))
            pp_sc = ctx.enter_context(
                tc.tile_pool(name="pp_sc", bufs=1, space="PSUM"))
            pp_uv = ctx.enter_context(
                tc.tile_pool(name="pp_uv", bufs=1, space="PSUM"))
            pp_sm = ctx.enter_context(
                tc.tile_pool(name="pp_sm", bufs=1, space="PSUM"))

            # ---- constants ----
            idf = const.tile([P, P], f16)
            make_identity(nc, idf)
            ones_row = const.tile([1, QPC], f16)
            nc.vector.memset(ones_row, 1.0)
            ones_col = const.tile([P, 1], f16)
            nc.vector.memset(ones_col, 1.0)
            zrow = const.tile([1, P], f16)
            nc.vector.memset(zrow, 0.0)
            zb = const.tile([P, 1], mybir.dt.float32)
            nc.vector.memset(zb, 0.0)
            hp = const.tile([P, 1], mybir.dt.float32)
            nc.vector.memset(hp, HALF_PI)

            # ---- inputs ----
            wvT = const.tile([P, 2, A], f16)
            wuT = const.tile([P, 2, A], f16)
            hT = const.tile([P, 2, S], f16)      # [d_p, d_chunk, s]
            h_sb = const.tile([P, 4, D], f16)    # [k_p, k_chunk, d]
            bu_sb = const.tile([1, A], f16)
            mbt = const.tile([P, 4], f32)
            bwt = const.tile([P, M], f32)
            ht_r = ht_ext.rearrange("(c p) s -> p c s", p=P)
            h_r = h_ext.rearrange("(t p) d -> p t d", p=P)
            # few big DMAs, one per engine queue (DGE issue cost dominates)
            nc.sync.dma_start(out=wvT,
                              in_=wvt_ext.rearrange("(c p) a -> p c a", p=P))
            nc.sync.dma_start(out=wuT,
                              in_=wut_ext.rearrange("(c p) a -> p c a", p=P))
            nc.scalar.dma_start(out=bu_sb, in_=bu_ext[:])

            nc.scalar.dma_start(out=bwt, in_=bwt_ext[:])
            nc.scalar.dma_start(out=mbt, in_=mbt_ext[:])
            nc.sync.dma_start(out=hT, in_=ht_r)
            nc.scalar.dma_start(out=h_sb, in_=h_r)

            # ---- psum tiles ----
            # one accumulation group per PSUM bank: a matmul group-start
            # resets the whole bank, so concurrent groups must not share one
            psT = pp_sc.tile([P, 4, 2 * QPC], f32)  # scores^T, kb -> own bank
            ps_uv = pp_uv.tile([P, 4 * QPC], f32, tag="uv")  # u bank0 | v bank1
            ps_sm = pp_sm.tile([P, 3, P], f32)      # sums cols + heater area

            # PE heater: garbage matmuls keep the clock-ramp window alive
            # while PE waits for DMA / feature maps.  Round-robin over the
            # unused upper halves of the score banks so they overlap; the
            # m=0 group-start resets those banks afterwards anyway.
            _hb = [0]
            def heat(n, dep):
                w = min(dep.shape[-1], P)
                for i in range(n):
                    _hb[0] = (_hb[0] + 1) % 4
                    nc.tensor.matmul(psT[0:w, _hb[0], QPC:QPC + w],
                                     dep[:, 0:w], dep[:, 0:w],
                                     start=False, stop=False,
                                     skip_group_check=True)

            heat(NH_INIT, idf)

            # ---- projections: ps_v = Wv h^T (+bu), ps_u = Wu h^T ----
            ps_v = ps_uv[:, 2 * QPC:3 * QPC]
            ps_u = ps_uv[:, 0:2 * QPC]
            nc.tensor.matmul(ps_v, bu_sb, ones_row, start=True, stop=False)
            for c in range(2):
                nc.tensor.matmul(ps_v, wvT[:, c, :], hT[:, c, 0:QPC],
                                 start=False, stop=(c == 1))
            for c in range(2):
                nc.tensor.matmul(ps_u, wuT[:, c, :], hT[:, c, :],
                                 start=(c == 0), stop=(c == 1))
            heat(NH_MID, idf)

            # ---- f16 copies + |.| ----
            uv = wrk.tile([P, W], f16)
            au = wrk.tile([P, W], f16)
            # f16 + clamp to +-4.2: keeps all base Sin args inside the ACT
            # table's accurate range (|arg| <= 4.2*0.993 ~ 4.17)
            nc.vector.tensor_scalar(out=uv[:, 0:S], in0=ps_u, scalar1=4.2,
                                    scalar2=-4.2, op0=ALU.min, op1=ALU.max)
            nc.vector.tensor_scalar(out=uv[:, S:W], in0=ps_v, scalar1=4.2,
                                    scalar2=-4.2, op0=ALU.min, op1=ALU.max)
            nc.vector.scalar_tensor_tensor(out=au, in0=uv, scalar=-1.0,
                                           in1=uv, op0=ALU.mult, op1=ALU.max)

            # ---- features: F[:, m, 0, :]=sin-ish, F[:, m, 1, :]=cos ----
            F = feat.tile([P, M, 2, W], f16)
            SQ = feat.tile([P, 6, W], f16)
            VS = feat.tile([P, M, 2, QPC], f16)

            def vscale(m):
                # v-side scale by beta*defer*Wt (gpsimd is ~30x too slow here)
                nc.vector.tensor_scalar(out=VS[:, m, :, :],
                                        in0=F[:, m, :, S:W],
                                        scalar1=bwt[:, m:m + 1], scalar2=None,
                                        op0=ALU.mult)

            def base_sins(i):
                nc.scalar.activation(F[:, i, 0, :], uv, SIN,
                                     bias=zb, scale=float(OMEGA[i]))
                nc.scalar.activation(F[:, i, 1, :], au, SIN,
                                     bias=hp, scale=float(-OMEGA[i]))

            def dbl(i, j, cc, sq_act=False):
                # derive tile i = double of tile j (cc = deferred factor of j)
                if sq_act:
                    nc.scalar.activation(SQ[:, j, :], F[:, j, 0, :], SQUARE,
                                         bias=zb, scale=1.0)
                else:
                    nc.vector.tensor_tensor(out=SQ[:, j, :],
                                            in0=F[:, j, 0, :],
                                            in1=F[:, j, 0, :], op=ALU.mult)
                nc.vector.tensor_tensor(out=F[:, i, 0, :], in0=F[:, j, 0, :],
                                        in1=F[:, j, 1, :], op=ALU.mult)
                nc.vector.tensor_scalar(out=F[:, i, 1, :], in0=SQ[:, j, :],
                                        scalar1=-2.0 * cc * cc, scalar2=1.0,
                                        op0=ALU.mult, op1=ALU.add)

            # deep-chain bases first so the DVE ladder starts early
            base_sins(1)
            vscale(1)
            dbl(4, 1, 1.0)        # 2b
            vscale(4)
            base_sins(2)
            dbl(6, 4, 2.0)   # 4b
            vscale(6)
            vscale(2)
            dbl(5, 2, 1.0)        # 2c
            vscale(5)
            base_sins(0)
            dbl(7, 5, 2.0)   # 4c
            vscale(7)
            vscale(0)
            dbl(3, 0, 1.0)   # 2a
            vscale(3)

            # ---- scores^T: psT[k, q] += sinU^T (bwt cosV) + cosU^T (bwt sinV)
            MS_ORDER = [1, 4, 6, 2, 5, 7, 0, 3]
            for mi, m in enumerate(MS_ORDER):
                for kb in range(4):
                    ks = slice(kb * P, (kb + 1) * P)
                    nc.tensor.matmul(psT[:, kb, 0:QPC], F[:, m, 0, ks],
                                     VS[:, m, 1, :],
                                     start=(mi == 0), stop=False)
                    nc.tensor.matmul(psT[:, kb, 0:QPC], F[:, m, 1, ks],
                                     VS[:, m, 0, :],
                                     start=False, stop=(mi == M - 1))

            # ---- masked softmax (mask = exp bias) + value matmuls ----
            expT = wrk.tile([P, 4, QPC], f16)
            for kb in range(4):
                nc.scalar.activation(expT[:, kb, :], psT[:, kb, 0:QPC], EXP,
                                     bias=mbt[:, kb:kb + 1], scale=1.0)
            ps_o = pp_uv.tile([P, 2, 2 * D], f32, tag="uv")
            # seed the sums bank once; both qb groups then accumulate
            nc.tensor.matmul(ps_sm[:, 0, 0:2], zrow, ones_row[:, 0:2],
                             start=True, stop=False, skip_group_check=True)
            for qb in range(2):
                qs = slice(qb * P, (qb + 1) * P)
                for kb in range(4):
                    nc.tensor.matmul(ps_o[:, qb, 0:D], expT[:, kb, qs],
                                     h_sb[:, kb, :],
                                     start=(kb == 0), stop=(kb == 3))
                    nc.tensor.matmul(ps_sm[:, 0, qb:qb + 1], expT[:, kb, qs],
                                     ones_col, start=False,
                                     stop=(qb == 1 and kb == 3),
                                     skip_group_check=True)
            rs = wrk.tile([P, 2], f32)
            nc.vector.reciprocal(rs, ps_sm[:, 0, 0:2])
            out_sb = wrk.tile([P, 2, D], f32)
            for qb in range(2):
                nc.vector.tensor_scalar(out=out_sb[:, qb, :],
                                        in0=ps_o[:, qb, 0:D],
                                        scalar1=rs[:, qb:qb + 1], scalar2=None,
                                        op0=ALU.mult)
                oq = nc.sync if qb == 0 else nc.scalar
                oq.dma_start(out=out_ext[qb * P:(qb + 1) * P, :],
                             in_=out_sb[:, qb, :])

    nc.compile()
    return nc


def kernel(hidden, mask, Wu, bu, Wv, Wt, bt):
    global _NC, LAST_EXEC_NS
    if _NC is None:
        _NC = _build_program()
    nc = _NC

    hidden = np.asarray(hidden, dtype=np.float32)
    mask = np.asarray(mask)
    Wu = np.asarray(Wu, dtype=np.float32)
    Wv = np.asarray(Wv, dtype=np.float32)
    Wt_v = np.asarray(Wt, dtype=np.float32).reshape(A)
    bu_v = np.asarray(bu, dtype=np.float32).reshape(A)

    WuT16 = np.ascontiguousarray(Wu.T.astype(np.float16))
    WvT16 = np.ascontiguousarray(Wv.T.astype(np.float16))
    bu16 = np.ascontiguousarray(bu_v.reshape(1, A).astype(np.float16))
    bwt8 = np.ascontiguousarray(
        (np.asarray(BETA) * np.asarray(DEFER))[None, :] * Wt_v[:, None]
    ).astype(np.float32)                                   # [A, M]

    in_maps = []
    for c in range(NCORES):
        b, half = divmod(c, 2)
        qoff = half * QPC
        hr = np.roll(hidden[b], -qoff, axis=0)
        h16 = np.ascontiguousarray(hr.astype(np.float16))
        ht16 = np.ascontiguousarray(hr.T.astype(np.float16))
        mb = np.where(np.asarray(mask[b]) < 1, MASK_NEG, 0.0).astype(np.float32)
        mbr = np.roll(mb, -qoff)                           # [S]
        mbt = np.ascontiguousarray(mbr.reshape(4, A).T)    # [A, 4] col = kblk
        in_maps.append({"h16": h16, "ht16": ht16, "wut16": WuT16,
                        "wvt16": WvT16, "bu16": bu16, "mbt": mbt,
                        "bwt8": bwt8})
    if TRACE:
        _ensure_ntff_hook()
    res = run_bass_kernel_spmd(nc, in_maps, list(range(NCORES)), trace=TRACE)
    LAST_EXEC_NS = res.exec_time_ns

    out = np.empty((B, S, D), dtype=np.float32)
    for c in range(NCORES):
        b, half = divmod(c, 2)
        qoff = half * QPC
        out[b, qoff:qoff + QPC] = res.results[c]["out"]
    return out


# revision 23
# speedup vs baseline: 1.2163x; 1.0331x over previous
"""Additive (Bahdanau-style) attention on 8 Trainium2 NeuronCores.

Math: scores[b,q,k] = Wt . tanh(u[b,k] + v[b,q]) + bt, masked softmax over k,
out = weights @ hidden.  (bt dropped: softmax is shift-invariant.)

tanh(x) on |x| <= 10.4 ~= sum_m beta_m sin(om_m x) with a *doubling-ladder*
frequency set {a, b, c, 2a, 2b, 2c, 4b, 4c}: only the 3 base frequencies are
evaluated with the ACT Sin table (all base args stay inside the table's
accurate range |arg| <~ 3.5, so NO range reduction is needed); the other 5
come from double-angle identities on the DVE:
    sin(2w) = 2 sin(w) cos(w)   (factor 2 deferred into the beta*Wt scale)
    cos(2w) = 1 - 2 sin(w)^2
cos phases come from cos(t) = sin(pi/2 - |t|) (always in table range).
The angle-addition identity sin(om(u+v)) = sinU cosV + cosU sinV then
factorizes the [Sq,Sk,A] tanh tensor into per-(a,k)/per-(a,q) f16 feature
maps plus PE matmuls contracting over A.

Scores are accumulated TRANSPOSED ([k, q]) so the key mask folds into the
softmax Exp's per-partition bias for free, and the exp weights feed the
value matmuls directly as stationary operands (no transposes/copies).

Sharding: core c -> batch b = c//2, query half qoff = (c%2)*256.  Each core's
inputs are rolled by -qoff so queries are always rows 0..255 (pure SPMD);
a key permutation is softmax-invariant when mask and values are permuted
consistently.
"""

import numpy as np

import concourse.bass as bass
import concourse.tile as tile
from concourse import bacc, mybir
from concourse.bass_utils import run_bass_kernel_spmd

# ---- problem constants (hardcoded; kernel.py must be self-contained) -------
B, S, D, A = 4, 512, 256, 128
QPC = 256          # queries per core
NCORES = 8
MASK_NEG = -30000.0
HALF_PI = float(np.pi / 2)

# ---- tanh ~= sum_m beta_m sin(om_m x), doubling ladder ---------------------
# m: 0..2 = bases {a,b,c} (ACT Sin), 3..5 = doubles {2a,2b,2c}, 6..7 = {4b,4c}
OMEGA = [0.19661398572560465, 0.7067266116223141, 0.992731100727015,
         0.3932279714512093, 1.4134532232446282, 1.98546220145403,
         2.8269064464892564, 3.97092440290806]
BETA = [1.0701015472411093, 0.2252177841876598, 0.1527837415896906,
        0.3031134355844352, 0.10362678834616496, 0.06220737550216762,
        0.0230515652310954, 0.005206629732025753]
# stored sin tiles carry a deferred factor (2 per doubling), folded into bwt
DEFER = [1.0, 1.0, 1.0, 2.0, 2.0, 2.0, 4.0, 4.0]
M = 8
NBASE = 3
W = S + QPC                      # concat feature width: [u(0:512) | v(512:768)]

NH_INIT = 10                     # PE heater matmuls during the DMA/proj fill
NH_MID = 14                      # heaters covering the Sin/feature gap

TRACE = False                    # test.py sets True for the profiled run
LAST_EXEC_NS = None


def _ensure_ntff_hook():
    """The agent image's `antenv` lacks `axon_hooks`; recreate the NTFF
    profiling hook registration (stub module wired to the ctypes profiler)."""
    import sys, types
    if "antenv.axon_hooks" in sys.modules:
        return
    mod = types.ModuleType("antenv.axon_hooks")
    _h = [None]
    mod.set_axon_ntff_profile_hook = lambda h: _h.__setitem__(0, h)
    mod.get_axon_ntff_profile_hook = lambda: _h[0]
    import antenv
    sys.modules["antenv.axon_hooks"] = mod
    antenv.axon_hooks = mod
    try:
        from trn_agent_boot.trn_boot import _ntff_profile_via_ctypes
        mod.set_axon_ntff_profile_hook(
            _ntff_profile_via_ctypes("/opt/axon/libaxon_pjrt.so"))
    except Exception:
        pass


# ---- device program --------------------------------------------------------
_NC = None


def _build_program():
    f32 = mybir.dt.float32
    f16 = mybir.dt.float16
    nc = bacc.Bacc("TRN2", target_bir_lowering=False, debug=False,
                   num_devices=NCORES)

    ht_ext = nc.dram_tensor("ht16", [D, S], f16, kind="ExternalInput").ap()
    h_ext = nc.dram_tensor("h16", [S, D + 1], f16, kind="ExternalInput").ap()
    wut_ext = nc.dram_tensor("wut16", [D, A], f16, kind="ExternalInput").ap()
    wvt_ext = nc.dram_tensor("wvt16", [D, A], f16, kind="ExternalInput").ap()
    bu_ext = nc.dram_tensor("bu16", [1, A], f16, kind="ExternalInput").ap()
    mbt_ext = nc.dram_tensor("mbt", [A, 4], f32, kind="ExternalInput").ap()
    bwt_ext = nc.dram_tensor("bwt8", [A, M], f32, kind="ExternalInput").ap()
    out_ext = nc.dram_tensor("out", [QPC, D], f32, kind="ExternalOutput").ap()

    from concourse.masks import make_identity
    P = 128
    SIN = mybir.ActivationFunctionType.Sin
    EXP = mybir.ActivationFunctionType.Exp
    COPY = mybir.ActivationFunctionType.Copy
    SQUARE = mybir.ActivationFunctionType.Square
    ALU = mybir.AluOpType

    with tile.TileContext(nc) as tc:
        import contextlib
        with contextlib.ExitStack() as ctx:
            const = ctx.enter_context(tc.tile_pool(name="const", bufs=1))
            feat = ctx.enter_context(tc.tile_pool(name="feat", bufs=1))
            wrk = ctx.enter_context(tc.tile_pool(name="wrk", bufs=1))
            pp_sc = ctx.enter_context(
                tc.tile_pool(name="pp_sc", bufs=1, space="PSUM"))
            pp_uv = ctx.enter_context(
                tc.tile_pool(name="pp_uv", bufs=1, space="PSUM"))

            # ---- constants ----
            idf = const.tile([P, P], f16)
            make_identity(nc, idf)
            ones_row = const.tile([1, QPC], f16)
            nc.vector.memset(ones_row, 1.0)
            zb = const.tile([P, 1], mybir.dt.float32)
            nc.vector.memset(zb, 0.0)
            hp = const.tile([P, 1], mybir.dt.float32)
            nc.vector.memset(hp, HALF_PI)

            # ---- inputs ----
            wvT = const.tile([P, 2, A], f16)
            wuT = const.tile([P, 2, A], f16)
            hT = const.tile([P, 2, S], f16)      # [d_p, d_chunk, s]
            h_sb = const.tile([P, 4, D + 1], f16)  # [k_p, k_chunk, d | ones]
            bu_sb = const.tile([1, A], f16)
            mbt = const.tile([P, 4], f32)
            bwt = const.tile([P, M], f32)
            ht_r = ht_ext.rearrange("(c p) s -> p c s", p=P)
            h_r = h_ext.rearrange("(t p) d -> p t d", p=P)
            # few big DMAs, one per engine queue (DGE issue cost dominates)
            nc.sync.dma_start(out=wvT,
                              in_=wvt_ext.rearrange("(c p) a -> p c a", p=P))
            nc.sync.dma_start(out=wuT,
                              in_=wut_ext.rearrange("(c p) a -> p c a", p=P))
            nc.scalar.dma_start(out=bu_sb, in_=bu_ext[:])

            nc.scalar.dma_start(out=bwt, in_=bwt_ext[:])
            nc.scalar.dma_start(out=mbt, in_=mbt_ext[:])
            nc.sync.dma_start(out=hT, in_=ht_r)
            nc.scalar.dma_start(out=h_sb, in_=h_r)

            # ---- psum tiles ----
            # one accumulation group per PSUM bank: a matmul group-start
            # resets the whole bank, so concurrent groups must not share one
            psT = pp_sc.tile([P, 4, 2 * QPC], f32)  # scores^T, kb -> own bank
            ps_uv = pp_uv.tile([P, 4 * QPC], f32, tag="uv")  # u bank0 | v bank1

            # PE heater: garbage matmuls keep the clock-ramp window alive
            # while PE waits for DMA / feature maps.  Round-robin over the
            # unused upper halves of the score banks so they overlap; the
            # m=0 group-start resets those banks afterwards anyway.
            _hb = [0]
            def heat(n, dep):
                w = min(dep.shape[-1], P)
                for i in range(n):
                    _hb[0] = (_hb[0] + 1) % 4
                    nc.tensor.matmul(psT[0:w, _hb[0], QPC:QPC + w],
                                     dep[:, 0:w], dep[:, 0:w],
                                     start=False, stop=False,
                                     skip_group_check=True)

            heat(NH_INIT, idf)

            # ---- projections: ps_v = Wv h^T (+bu), ps_u = Wu h^T ----
            ps_v = ps_uv[:, 2 * QPC:3 * QPC]
            ps_u = ps_uv[:, 0:2 * QPC]
            nc.tensor.matmul(ps_v, bu_sb, ones_row, start=True, stop=False)
            for c in range(2):
                nc.tensor.matmul(ps_v, wvT[:, c, :], hT[:, c, 0:QPC],
                                 start=False, stop=(c == 1))
            for c in range(2):
                nc.tensor.matmul(ps_u, wuT[:, c, :], hT[:, c, :],
                                 start=(c == 0), stop=(c == 1))
            heat(NH_MID, idf)

            # ---- f16 copies + |.| ----
            uv = wrk.tile([P, W], f16)
            au = wrk.tile([P, W], f16)
            # f16 + clamp to +-4.2: keeps all base Sin args inside the ACT
            # table's accurate range (|arg| <= 4.2*0.993 ~ 4.17)
            nc.vector.tensor_scalar(out=uv[:, 0:S], in0=ps_u, scalar1=4.2,
                                    scalar2=-4.2, op0=ALU.min, op1=ALU.max)
            nc.vector.tensor_scalar(out=uv[:, S:W], in0=ps_v, scalar1=4.2,
                                    scalar2=-4.2, op0=ALU.min, op1=ALU.max)
            nc.vector.scalar_tensor_tensor(out=au, in0=uv, scalar=-1.0,
                                           in1=uv, op0=ALU.mult, op1=ALU.max)

            # ---- features: F[:, m, 0, :]=sin-ish, F[:, m, 1, :]=cos ----
            F = feat.tile([P, M, 2, W], f16)
            SQ = feat.tile([P, 6, W], f16)
            VS = feat.tile([P, M, 2, QPC], f16)

            def vscale(m):
                # v-side scale by beta*defer*Wt (gpsimd is ~30x too slow here)
                nc.vector.tensor_scalar(out=VS[:, m, :, :],
                                        in0=F[:, m, :, S:W],
                                        scalar1=bwt[:, m:m + 1], scalar2=None,
                                        op0=ALU.mult)

            def base_sins(i):
                nc.scalar.activation(F[:, i, 0, :], uv, SIN,
                                     bias=zb, scale=float(OMEGA[i]))
                nc.scalar.activation(F[:, i, 1, :], au, SIN,
                                     bias=hp, scale=float(-OMEGA[i]))

            def dbl(i, j, cc, sq_act=False):
                # derive tile i = double of tile j (cc = deferred factor of j)
                if sq_act:
                    nc.scalar.activation(SQ[:, j, :], F[:, j, 0, :], SQUARE,
                                         bias=zb, scale=1.0)
                else:
                    nc.vector.tensor_tensor(out=SQ[:, j, :],
                                            in0=F[:, j, 0, :],
                                            in1=F[:, j, 0, :], op=ALU.mult)
                nc.vector.tensor_tensor(out=F[:, i, 0, :], in0=F[:, j, 0, :],
                                        in1=F[:, j, 1, :], op=ALU.mult)
                nc.vector.tensor_scalar(out=F[:, i, 1, :], in0=SQ[:, j, :],
                                        scalar1=-2.0 * cc * cc, scalar2=1.0,
                                        op0=ALU.mult, op1=ALU.add)

            # deep-chain bases first so the DVE ladder starts early
            base_sins(1)
            vscale(1)
            dbl(4, 1, 1.0)        # 2b
            vscale(4)
            base_sins(2)
            dbl(6, 4, 2.0)   # 4b
            vscale(6)
            vscale(2)
            dbl(5, 2, 1.0)        # 2c
            vscale(5)
            base_sins(0)
            dbl(7, 5, 2.0)   # 4c
            vscale(7)
            vscale(0)
            dbl(3, 0, 1.0)   # 2a
            vscale(3)

            # ---- scores^T: psT[k, q] += sinU^T (bwt cosV) + cosU^T (bwt sinV)
            MS_ORDER = [1, 4, 6, 2, 5, 7, 0, 3]
            for mi, m in enumerate(MS_ORDER):
                for kb in range(4):
                    ks = slice(kb * P, (kb + 1) * P)
                    nc.tensor.matmul(psT[:, kb, 0:QPC], F[:, m, 0, ks],
                                     VS[:, m, 1, :],
                                     start=(mi == 0), stop=False)
                    nc.tensor.matmul(psT[:, kb, 0:QPC], F[:, m, 1, ks],
                                     VS[:, m, 0, :],
                                     start=False, stop=(mi == M - 1))

            # ---- masked softmax (mask = exp bias) + value matmuls ----
            expT = wrk.tile([P, 4, QPC], f16)
            for kb in range(4):
                nc.scalar.activation(expT[:, kb, :], psT[:, kb, 0:QPC], EXP,
                                     bias=mbt[:, kb:kb + 1], scale=1.0)
            ps_o = pp_uv.tile([P, 2, 2 * D], f32, tag="uv")
            # value matmuls; the ones column of h accumulates the softmax sums
            for kb in range(4):
                for qb in range(2):
                    qs = slice(qb * P, (qb + 1) * P)
                    nc.tensor.matmul(ps_o[:, qb, 0:D + 1], expT[:, kb, qs],
                                     h_sb[:, kb, :],
                                     start=(kb == 0), stop=(kb == 3))
            rs = wrk.tile([P, 2], f32)
            for qb in range(2):
                nc.vector.reciprocal(rs[:, qb:qb + 1], ps_o[:, qb, D:D + 1])
            out_sb = wrk.tile([P, 2, D], f32)
            for qb in range(2):
                nc.vector.tensor_scalar(out=out_sb[:, qb, :],
                                        in0=ps_o[:, qb, 0:D],
                                        scalar1=rs[:, qb:qb + 1], scalar2=None,
                                        op0=ALU.mult)
                oq = nc.sync if qb == 0 else nc.scalar
                oq.dma_start(out=out_ext[qb * P:(qb + 1) * P, :],
                             in_=out_sb[:, qb, :])

    nc.compile()
    return nc


def kernel(hidden, mask, Wu, bu, Wv, Wt, bt):
    global _NC, LAST_EXEC_NS
    if _NC is None:
        _NC = _build_program()
    nc = _NC

    hidden = np.asarray(hidden, dtype=np.float32)
    mask = np.asarray(mask)
    Wu = np.asarray(Wu, dtype=np.float32)
    Wv = np.asarray(Wv, dtype=np.float32)
    Wt_v = np.asarray(Wt, dtype=np.float32).reshape(A)
    bu_v = np.asarray(bu, dtype=np.float32).reshape(A)

    WuT16 = np.ascontiguousarray(Wu.T.astype(np.float16))
    WvT16 = np.ascontiguousarray(Wv.T.astype(np.float16))
    bu16 = np.ascontiguousarray(bu_v.reshape(1, A).astype(np.float16))
    bwt8 = np.ascontiguousarray(
        (np.asarray(BETA) * np.asarray(DEFER))[None, :] * Wt_v[:, None]
    ).astype(np.float32)                                   # [A, M]

    in_maps = []
    for c in range(NCORES):
        b, half = divmod(c, 2)
        qoff = half * QPC
        hr = np.roll(hidden[b], -qoff, axis=0)
        h16 = np.ascontiguousarray(
            np.concatenate([hr, np.ones((S, 1), np.float32)], axis=1
                           ).astype(np.float16))
        ht16 = np.ascontiguousarray(hr.T.astype(np.float16))
        mb = np.where(np.asarray(mask[b]) < 1, MASK_NEG, 0.0).astype(np.float32)
        mbr = np.roll(mb, -qoff)                           # [S]
        mbt = np.ascontiguousarray(mbr.reshape(4, A).T)    # [A, 4] col = kblk
        in_maps.append({"h16": h16, "ht16": ht16, "wut16": WuT16,
                        "wvt16": WvT16, "bu16": bu16, "mbt": mbt,
                        "bwt8": bwt8})
    if TRACE:
        _ensure_ntff_hook()
    res = run_bass_kernel_spmd(nc, in_maps, list(range(NCORES)), trace=TRACE)
    LAST_EXEC_NS = res.exec_time_ns

    out = np.empty((B, S, D), dtype=np.float32)
    for c in range(NCORES):
        b, half = divmod(c, 2)
        qoff = half * QPC
        out[b, qoff:qoff + QPC] = res.results[c]["out"]
    return out
